# revision 1
# baseline (speedup 1.0000x reference)
"""Fused attention block (QKV proj -> softmax attention -> out proj -> residual+LN)
for B=4, S=2048, D=512, H=8, DH=64 on 8 TRN2 NeuronCores.

Sharding: token-parallel. Core c handles batch b=c//2, query tokens
[(c%2)*1024, (c%2+1)*1024) of that batch. Each core redundantly computes K/V
for its batch's full 2048-token sequence (cheaper than an AllGather), runs
flash-style attention fully on-chip (the 537MB score tensor never touches HBM),
and writes its own disjoint 1024x512 slice of the output. Zero collectives.

All matmuls run in bf16 (fp32 PSUM accumulation); fp32 matmul on TRN2 is 4x
slower. Host pre-transposes x and the weights so the device never transposes.
Softmax denominators come from an extra ones-column appended to V (row 64 of
the ctx^T accumulation), so no partition-axis reduction is ever needed.
"""

import os
import sys

import numpy as np

for _p in ("/opt/trn_rl_repo",):
    if os.path.isdir(_p) and _p not in sys.path:
        sys.path.insert(0, _p)

import ml_dtypes

import concourse.bacc as bacc
import concourse.bass as bass
import concourse.tile as tile
from concourse import mybir
from concourse.bass_utils import run_bass_kernel_spmd

BF16 = mybir.dt.bfloat16
F32 = mybir.dt.float32
AF = mybir.ActivationFunctionType
ALU = mybir.AluOpType

P = 128        # partitions
D = 512        # hidden dim
DH = 64        # head dim
H = 8          # heads
S = 2048       # tokens per batch element
TQ = 1024      # query tokens per core
B = 4
NCORES = 8
EPS = 1e-5

TRACE = False
LAST_RESULTS = None
_NC_CACHE = None


def _build():
    nc = bacc.Bacc()

    xt = nc.declare_dram_parameter("xt", [D, S], BF16, isOutput=False)      # x[b].T
    xtq = nc.declare_dram_parameter("xtq", [D, TQ], BF16, isOutput=False)   # x_local.T
    xres = nc.declare_dram_parameter("xres", [TQ, D], F32, isOutput=False)  # x_local + bo
    wqt = nc.declare_dram_parameter("wqt", [D, D], BF16, isOutput=False)    # Wq.T
    wkt = nc.declare_dram_parameter("wkt", [D, D], BF16, isOutput=False)
    wvt = nc.declare_dram_parameter("wvt", [D, D], BF16, isOutput=False)
    wot = nc.declare_dram_parameter("wot", [D, D], BF16, isOutput=False)
    bqp = nc.declare_dram_parameter("bq", [P, 4], F32, isOutput=False)   # bq.reshape(4,128).T
    bkp = nc.declare_dram_parameter("bk", [P, 4], F32, isOutput=False)
    bvp = nc.declare_dram_parameter("bv", [P, D], F32, isOutput=False)   # host-broadcast
    gmp = nc.declare_dram_parameter("gamma", [P, D], F32, isOutput=False)
    btp = nc.declare_dram_parameter("beta", [P, D], F32, isOutput=False)
    outp = nc.declare_dram_parameter("out", [TQ, D], F32, isOutput=True)

    with tile.TileContext(nc) as tc:
        with (
            tc.tile_pool(name="big", bufs=1) as big,
            tc.tile_pool(name="work", bufs=3) as work,
            tc.tile_pool(name="ps_st", bufs=2, space="PSUM") as ps_st,
            tc.tile_pool(name="ps_ctx", bufs=2, space="PSUM") as ps_ctx,
            tc.tile_pool(name="ps_mm", bufs=2, space="PSUM") as ps_mm,
        ):
            # ---------------- loads ----------------
            xt_sb = big.tile([P, 4, S], BF16)
            xtq_sb = big.tile([P, 4, TQ], BF16)
            wq_sb = big.tile([P, 4, D], BF16)
            wk_sb = big.tile([P, 4, D], BF16)
            wv_sb = big.tile([P, 4, D], BF16)
            wo_sb = big.tile([P, 4, D], BF16)
            for c in range(4):
                nc.sync.dma_start(out=xt_sb[:, c, :], in_=xt[c * P:(c + 1) * P, :])
                nc.sync.dma_start(out=xtq_sb[:, c, :], in_=xtq[c * P:(c + 1) * P, :])
                nc.sync.dma_start(out=wq_sb[:, c, :], in_=wqt[c * P:(c + 1) * P, :])
                nc.sync.dma_start(out=wk_sb[:, c, :], in_=wkt[c * P:(c + 1) * P, :])
                nc.sync.dma_start(out=wv_sb[:, c, :], in_=wvt[c * P:(c + 1) * P, :])
                nc.sync.dma_start(out=wo_sb[:, c, :], in_=wot[c * P:(c + 1) * P, :])
            xres_sb = big.tile([P, 8, D], F32)
            for i in range(8):
                nc.sync.dma_start(out=xres_sb[:, i, :], in_=xres[i * P:(i + 1) * P, :])
            bq_sb = big.tile([P, 4], F32)
            bk_sb = big.tile([P, 4], F32)
            nc.sync.dma_start(out=bq_sb[:, :], in_=bqp[:, :])
            nc.sync.dma_start(out=bk_sb[:, :], in_=bkp[:, :])
            gm_sb = big.tile([P, D], F32)
            bt_sb = big.tile([P, D], F32)
            bv_sb = big.tile([P, D], F32)
            nc.sync.dma_start(out=gm_sb[:, :], in_=gmp[:, :])
            nc.sync.dma_start(out=bt_sb[:, :], in_=btp[:, :])
            nc.sync.dma_start(out=bv_sb[:, :], in_=bvp[:, :])
            eps_sb = big.tile([P, 1], F32)
            nc.vector.memset(eps_sb[:, :], EPS)

            # V augmented with a ones column per head: [tok, (h, 64 dims + 1)]
            vaug = big.tile([P, 16, H * 65], BF16)
            nc.vector.memset(
                vaug[:, :, :].rearrange("p c (h e) -> p c h e", e=65)[:, :, :, 64:65],
                1.0,
            )

            qt_all = big.tile([P, 4, TQ], BF16)   # Q^T  [dq, tq]
            kt_all = big.tile([P, 4, S], BF16)    # K^T  [dk, t]
            ctxT = big.tile([P, 4, TQ], BF16)     # ctx^T [dv, tq]
            y_all = big.tile([P, 8, D], F32)      # proj + residual
            mv_all = big.tile([P, 8, 2], F32)     # (mean, var) per token tile
            rstd_all = big.tile([P, 8], F32)

            # ---------------- QKV projections ----------------
            # Q^T[m*128+p, t] = sum_d Wq[m*128+p, d] * x[t, d]
            for m in range(4):
                for t2 in range(2):
                    ps = ps_mm.tile([P, 512], F32, tag="mm")
                    for kc in range(4):
                        nc.tensor.matmul(
                            ps[:, :],
                            lhsT=wq_sb[:, kc, m * P:(m + 1) * P],
                            rhs=xtq_sb[:, kc, t2 * 512:(t2 + 1) * 512],
                            start=(kc == 0),
                            stop=(kc == 3),
                        )
                    nc.scalar.activation(
                        out=qt_all[:, m, t2 * 512:(t2 + 1) * 512],
                        in_=ps[:, :],
                        func=AF.Identity,
                        bias=bq_sb[:, m:m + 1],
                        scale=1.0,
                    )
            for m in range(4):
                for t4 in range(4):
                    ps = ps_mm.tile([P, 512], F32, tag="mm")
                    for kc in range(4):
                        nc.tensor.matmul(
                            ps[:, :],
                            lhsT=wk_sb[:, kc, m * P:(m + 1) * P],
                            rhs=xt_sb[:, kc, t4 * 512:(t4 + 1) * 512],
                            start=(kc == 0),
                            stop=(kc == 3),
                        )
                    nc.scalar.activation(
                        out=kt_all[:, m, t4 * 512:(t4 + 1) * 512],
                        in_=ps[:, :],
                        func=AF.Identity,
                        bias=bk_sb[:, m:m + 1],
                        scale=1.0,
                    )
            # V natural [tok, dv], written strided into vaug (+bias)
            for t16 in range(16):
                ps = ps_mm.tile([P, 512], F32, tag="mm")
                for kc in range(4):
                    nc.tensor.matmul(
                        ps[:, :],
                        lhsT=xt_sb[:, kc, t16 * P:(t16 + 1) * P],
                        rhs=wv_sb[:, kc, :],
                        start=(kc == 0),
                        stop=(kc == 3),
                    )
                nc.vector.tensor_add(
                    out=vaug[:, t16, :].rearrange("p (h e) -> p h e", e=65)[:, :, 0:64],
                    in0=ps[:, :].rearrange("p (h e) -> p h e", e=64),
                    in1=bv_sb[:, :].rearrange("p (h e) -> p h e", e=64),
                )

            # ---------------- attention ----------------
            for h in range(H):
                po = (h % 2) * 64
                chn = h // 2
                cx0 = ps_ctx.tile([65, 512], F32, tag="cx")
                cx1 = ps_ctx.tile([65, 512], F32, tag="cx")
                for kc in range(16):
                    st = ps_st.tile([P, TQ], F32, tag="st")
                    kslice = kt_all[po:po + 64, chn, kc * P:(kc + 1) * P]
                    nc.tensor.matmul(
                        st[:, 0:512],
                        lhsT=kslice,
                        rhs=qt_all[po:po + 64, chn, 0:512],
                        start=True,
                        stop=True,
                    )
                    nc.tensor.matmul(
                        st[:, 512:1024],
                        lhsT=kslice,
                        rhs=qt_all[po:po + 64, chn, 512:1024],
                        start=True,
                        stop=True,
                    )
                    pr = work.tile([P, TQ], BF16, tag="probs")
                    nc.scalar.activation(
                        out=pr[:, :], in_=st[:, :], func=AF.Exp, scale=0.125
                    )
                    vh = vaug[:, kc, h * 65:(h + 1) * 65]
                    nc.tensor.matmul(
                        cx0[:, :], lhsT=vh, rhs=pr[:, 0:512],
                        start=(kc == 0), stop=(kc == 15),
                    )
                    nc.tensor.matmul(
                        cx1[:, :], lhsT=vh, rhs=pr[:, 512:1024],
                        start=(kc == 0), stop=(kc == 15),
                    )
                for qt2, cx in ((0, cx0), (1, cx1)):
                    rec = work.tile([1, 512], F32, tag="rec")
                    nc.vector.reciprocal(rec[:, :], cx[64:65, :])
                    recb = work.tile([64, 512], F32, tag="recb")
                    nc.gpsimd.partition_broadcast(recb[:, :], rec[:, :])
                    nc.vector.tensor_mul(
                        out=ctxT[po:po + 64, chn, qt2 * 512:(qt2 + 1) * 512],
                        in0=cx[0:64, :],
                        in1=recb[:, :],
                    )

            # ---------------- out proj + residual + stats ----------------
            for t8 in range(8):
                ps = ps_mm.tile([P, D], F32, tag="mm")
                for c in range(4):
                    nc.tensor.matmul(
                        ps[:, :],
                        lhsT=ctxT[:, c, t8 * P:(t8 + 1) * P],
                        rhs=wo_sb[:, c, :],
                        start=(c == 0),
                        stop=(c == 3),
                    )
                nc.vector.tensor_add(
                    out=y_all[:, t8, :], in0=ps[:, :], in1=xres_sb[:, t8, :]
                )
                stt = work.tile([P, 6], F32, tag="bnst")
                nc.vector.bn_stats(out=stt[:, :], in_=y_all[:, t8, :])
                nc.vector.bn_aggr(out=mv_all[:, t8, :], in_=stt[:, :])

            # ---------------- layernorm ----------------
            std_all = work.tile([P, 8], F32, tag="std")
            nc.scalar.activation(
                out=std_all[:, :],
                in_=mv_all[:, :, 1],
                func=AF.Sqrt,
                bias=eps_sb[:, :],
                scale=1.0,
            )
            nc.vector.reciprocal(rstd_all[:, :], std_all[:, :])
            for t8 in range(8):
                tmp = work.tile([P, D], F32, tag="lntmp")
                nc.vector.scalar_tensor_tensor(
                    out=tmp[:, :],
                    in0=y_all[:, t8, :],
                    scalar=mv_all[:, t8, 0:1],
                    in1=gm_sb[:, :],
                    op0=ALU.subtract,
                    op1=ALU.mult,
                )
                fin = work.tile([P, D], F32, tag="lnfin")
                nc.vector.scalar_tensor_tensor(
                    out=fin[:, :],
                    in0=tmp[:, :],
                    scalar=rstd_all[:, t8:t8 + 1],
                    in1=bt_sb[:, :],
                    op0=ALU.mult,
                    op1=ALU.add,
                )
                nc.sync.dma_start(out=outp[t8 * P:(t8 + 1) * P, :], in_=fin[:, :])

    nc.compile()
    return nc


def _get_nc():
    global _NC_CACHE
    if _NC_CACHE is None:
        _NC_CACHE = _build()
    return _NC_CACHE


def kernel(x, Wq, bq, Wk, bk, Wv, bv, Wo, bo, gamma, beta):
    global LAST_RESULTS
    bf = ml_dtypes.bfloat16
    x = np.asarray(x, np.float32)
    bo = np.asarray(bo, np.float32)
    wqt_n = np.ascontiguousarray(np.asarray(Wq, np.float32).T).astype(bf)
    wkt_n = np.ascontiguousarray(np.asarray(Wk, np.float32).T).astype(bf)
    wvt_n = np.ascontiguousarray(np.asarray(Wv, np.float32).T).astype(bf)
    wot_n = np.ascontiguousarray(np.asarray(Wo, np.float32).T).astype(bf)
    bq_n = np.ascontiguousarray(np.asarray(bq, np.float32).reshape(4, P).T)
    bk_n = np.ascontiguousarray(np.asarray(bk, np.float32).reshape(4, P).T)
    bv_n = np.ascontiguousarray(
        np.broadcast_to(np.asarray(bv, np.float32)[None, :], (P, D)))
    gm_n = np.ascontiguousarray(
        np.broadcast_to(np.asarray(gamma, np.float32)[None, :], (P, D)))
    bt_n = np.ascontiguousarray(
        np.broadcast_to(np.asarray(beta, np.float32)[None, :], (P, D)))

    in_maps = []
    for c in range(NCORES):
        b = c // 2
        par = c % 2
        xb = x[b]                               # [S, D]
        xloc = xb[par * TQ:(par + 1) * TQ]      # [TQ, D]
        in_maps.append({
            "xt": np.ascontiguousarray(xb.T).astype(bf),
            "xtq": np.ascontiguousarray(xloc.T).astype(bf),
            "xres": np.ascontiguousarray(xloc + bo[None, :], dtype=np.float32),
            "wqt": wqt_n, "wkt": wkt_n, "wvt": wvt_n, "wot": wot_n,
            "bq": bq_n, "bk": bk_n, "bv": bv_n,
            "gamma": gm_n, "beta": bt_n,
        })

    nc = _get_nc()
    res = run_bass_kernel_spmd(nc, in_maps, core_ids=list(range(NCORES)), trace=TRACE)
    LAST_RESULTS = res

    outf = np.empty((B, S, D), np.float32)
    for c in range(NCORES):
        b = c // 2
        par = c % 2
        outf[b, par * TQ:(par + 1) * TQ, :] = res.results[c]["out"]
    return outf



# revision 5
# speedup vs baseline: 1.1666x; 1.1666x over previous
"""Fused attention block (QKV proj -> softmax attention -> out proj -> residual+LN)
for B=4, S=2048, D=512, H=8, DH=64 on 8 TRN2 NeuronCores.

v2: fp8 everywhere it pays. All big matmuls run fp8e4 with DoubleRow perf mode
(2 k-tiles per pass, 0.5 cycles/row): QKV/out projections contract 512 in two
256-deep passes; scores contract dh=64 as 2x32 (host pre-permutes Wq/Wk rows so
each head's dh splits across two 32-partition k-tiles); ctx contracts keys as
2x128 (probs pairs). Probs are fp8e5m2 (range to 57344 -> no max-subtraction
needed at score|max| ~9.6 after the 1/8 scale). Exp is split across the scalar
engine (true Exp activation) and DVE (Schraudolph bit-trick: affine fp32->uint8
convert whose bits, read as e5m2, approximate exp) per a static pattern.
Softmax denominators come from a ones-column in the V tile (row 64 of ctx^T);
reciprocal on DVE, partition-broadcast on gpsimd, normalize-mul on DVE.
Residual add is folded into the out-projection as an identity-weights f32r
matmul; rstd = exp(-0.5*ln(var+eps)) keeps the scalar engine on one activation
table (exp/ln/identity/copy) for the whole kernel. LayerNorm scalar_tensor_
tensor muls alternate DVE/gpsimd.

Sharding: token-parallel, zero collectives. Core c handles batch b=c//2, query
tokens [(c%2)*1024, (c%2+1)*1024); K/V for the full 2048-token sequence are
computed redundantly per core (cheaper than an AllGather).
"""

import os
import sys

import numpy as np

for _p in ("/opt/trn_rl_repo",):
    if os.path.isdir(_p) and _p not in sys.path:
        sys.path.insert(0, _p)

import ml_dtypes

import concourse.bacc as bacc
import concourse.tile as tile
from concourse import mybir
from concourse.bass_utils import run_bass_kernel_spmd

BF16 = mybir.dt.bfloat16
F32 = mybir.dt.float32
F32R = mybir.dt.float32r
E4 = mybir.dt.float8e4
E5 = mybir.dt.float8e5
U8 = mybir.dt.uint8
AF = mybir.ActivationFunctionType
ALU = mybir.AluOpType
DR = mybir.MatmulPerfMode.DoubleRow

P = 128
D = 512
DH = 64
H = 8
S = 2048
TQ = 1024
B = 4
NCORES = 8
EPS = 1e-5

# Schraudolph e5m2 exp: bits = st*SCHRAU_A + SCHRAU_B, bits read as e5m2.
# A = 0.125 * (2^2 / ln 2); B = 15*4 + 0.2 (0.5 trunc hedge - 0.3 bias opt).
SCHRAU_A = 0.125 * (4.0 / np.log(2.0))
SCHRAU_B = 60.2

# Per-kc exp engine: 'S' scalar true exp, 'D' DVE Schraudolph. 10 S / 6 D.
EXP_PATTERN = "SSDSDSSDSDSSDSDS"

TRACE = False
LAST_RESULTS = None
_NC_CACHE = None


def _build():
    nc = bacc.Bacc()

    xt8d = nc.declare_dram_parameter("xt8", [P, 2, 2, S], E4, isOutput=False)
    xtq8d = nc.declare_dram_parameter("xtq8", [P, 2, 2, TQ], E4, isOutput=False)
    wq8d = nc.declare_dram_parameter("wq8", [P, 2, 2, D], E4, isOutput=False)
    wk8d = nc.declare_dram_parameter("wk8", [P, 2, 2, D], E4, isOutput=False)
    wv8d = nc.declare_dram_parameter("wv8", [P, 2, 2, D], E4, isOutput=False)
    wo8d = nc.declare_dram_parameter("wo8", [P, 2, 2, D], E4, isOutput=False)
    bqd = nc.declare_dram_parameter("bqp", [P, 4], F32, isOutput=False)
    bkd = nc.declare_dram_parameter("bkp", [P, 4], F32, isOutput=False)
    xresd = nc.declare_dram_parameter("xres", [P, 8, D], BF16, isOutput=False)
    idd = nc.declare_dram_parameter("id128", [P, P], BF16, isOutput=False)
    gmd = nc.declare_dram_parameter("gamma", [P, D], F32, isOutput=False)
    btd = nc.declare_dram_parameter("beta", [P, D], F32, isOutput=False)
    outd = nc.declare_dram_parameter("out", [TQ, D], F32, isOutput=True)

    with tile.TileContext(nc) as tc:
        with (
            tc.tile_pool(name="big", bufs=1) as big,
            tc.tile_pool(name="work", bufs=3) as work,
            tc.tile_pool(name="ps_st", bufs=3, space="PSUM") as ps_st,
            tc.tile_pool(name="ps_cx", bufs=1, space="PSUM") as ps_cx,
        ):
            # ---------------- loads ----------------
            wk_sb = big.tile([P, 2, 2, D], E4)
            wq_sb = big.tile([P, 2, 2, D], E4)
            wv_sb = big.tile([P, 2, 2, D], E4)
            wo_sb = big.tile([P, 2, 2, D], E4)
            xt_sb = big.tile([P, 2, 2, S], E4)
            xtq_sb = big.tile([P, 2, 2, TQ], E4)
            nc.sync.dma_start(out=wk_sb[:, :, :, :], in_=wk8d[:, :, :, :])
            nc.sync.dma_start(out=xt_sb[:, :, :, :], in_=xt8d[:, :, :, :])
            nc.sync.dma_start(out=wq_sb[:, :, :, :], in_=wq8d[:, :, :, :])
            nc.sync.dma_start(out=xtq_sb[:, :, :, :], in_=xtq8d[:, :, :, :])
            nc.sync.dma_start(out=wv_sb[:, :, :, :], in_=wv8d[:, :, :, :])
            bq_sb = big.tile([P, 4], F32)
            bk_sb = big.tile([P, 4], F32)
            nc.sync.dma_start(out=bq_sb[:, :], in_=bqd[:, :])
            nc.sync.dma_start(out=bk_sb[:, :], in_=bkd[:, :])
            wo_ld = nc.sync.dma_start(out=wo_sb[:, :, :, :], in_=wo8d[:, :, :, :])
            xres_sb = big.tile([P, 8, D], BF16)
            id_sb = big.tile([P, P], BF16)
            gm_sb = big.tile([P, D], F32)
            bt_sb = big.tile([P, D], F32)
            nc.sync.dma_start(out=xres_sb[:, :, :], in_=xresd[:, :, :])
            nc.sync.dma_start(out=id_sb[:, :], in_=idd[:, :])
            nc.sync.dma_start(out=gm_sb[:, :], in_=gmd[:, :])
            nc.sync.dma_start(out=bt_sb[:, :], in_=btd[:, :])
            eps_sb = big.tile([P, 1], F32)
            nc.gpsimd.memset(eps_sb[:, :], EPS)

            qt8 = big.tile([P, 4, TQ], E4)
            kt8 = big.tile([P, 4, S], E4)
            vaug = big.tile([P, 16, H, 80], E5)  # 80-col head stride: 16B-aligned for dual-fp8 ldweights
            nc.gpsimd.memset(vaug[:, :, :, 64:65], 1.0)
            pr8 = big.tile([P, 16, TQ], E5)
            ctxT8 = big.tile([P, 2, 2, TQ], E4)
            y_sb = big.tile([P, 8, D], F32)
            mv_all = big.tile([P, 8, 2], F32)
            lnu = big.tile([P, 8], F32)
            rstd_all = big.tile([P, 8], F32)

            copy_rr = [0]  # round-robin engine for PSUM->SBUF proj copies

            def proj_copy(dst, src, bias_ap):
                if copy_rr[0] % 2 == 0:
                    if bias_ap is None:
                        nc.scalar.activation(out=dst, in_=src, func=AF.Copy)
                    else:
                        nc.scalar.activation(
                            out=dst, in_=src, func=AF.Identity, bias=bias_ap
                        )
                else:
                    nc.vector.tensor_scalar(
                        dst, src, bias_ap if bias_ap is not None else 0.0,
                        None, ALU.add,
                    )
                copy_rr[0] += 1

            # ---------------- K/Q/V projections (fp8 DoubleRow) ----------------
            for mb in range(4):
                for j2 in range(2):
                    ps = ps_st.tile([P, 1024], F32, tag="st")
                    for half in range(2):
                        t4 = 2 * j2 + half
                        for a in range(2):
                            nc.tensor.matmul(
                                ps[:, half * 512:(half + 1) * 512],
                                lhsT=wk_sb[:, a, :, mb * P:(mb + 1) * P],
                                rhs=xt_sb[:, a, :, t4 * 512:(t4 + 1) * 512],
                                start=(a == 0),
                                stop=(a == 1),
                                perf_mode=DR,
                            )
                    proj_copy(
                        kt8[:, mb, 2 * j2 * 512:(2 * j2 + 2) * 512],
                        ps[:, :],
                        bk_sb[:, mb:mb + 1],
                    )
            for mb in range(4):
                ps = ps_st.tile([P, 1024], F32, tag="st")
                for half in range(2):
                    for a in range(2):
                        nc.tensor.matmul(
                            ps[:, half * 512:(half + 1) * 512],
                            lhsT=wq_sb[:, a, :, mb * P:(mb + 1) * P],
                            rhs=xtq_sb[:, a, :, half * 512:(half + 1) * 512],
                            start=(a == 0),
                            stop=(a == 1),
                            perf_mode=DR,
                        )
                proj_copy(qt8[:, mb, :], ps[:, :], bq_sb[:, mb:mb + 1])
            for j in range(8):
                ps = ps_st.tile([P, 1024], F32, tag="st")
                for half in range(2):
                    t16 = 2 * j + half
                    for a in range(2):
                        nc.tensor.matmul(
                            ps[:, half * 512:(half + 1) * 512],
                            lhsT=xt_sb[:, a, :, t16 * P:(t16 + 1) * P],
                            rhs=wv_sb[:, a, :, :],
                            start=(a == 0),
                            stop=(a == 1),
                            perf_mode=DR,
                        )
                proj_copy(
                    vaug[:, 2 * j:2 * j + 2, :, 0:64],
                    ps[:, :].rearrange("p (two h e) -> p two h e", two=2, h=H),
                    None,
                )

            # ---------------- attention ----------------
            pr8u = pr8.bitcast(U8)
            for h in range(H):
                sub = h % 4
                chn = h // 4
                mb2 = 2 * chn
                cx = ps_cx.tile([65, 1024], F32, tag="cx")
                for kp in range(8):
                    for j in range(2):
                        kc = 2 * kp + j
                        st = ps_st.tile([P, 1024], F32, tag="st")
                        for half in range(2):
                            nc.tensor.matmul(
                                st[:, half * 512:(half + 1) * 512],
                                lhsT=kt8[sub * 32:(sub + 1) * 32, mb2:mb2 + 2,
                                         kc * P:(kc + 1) * P],
                                rhs=qt8[sub * 32:(sub + 1) * 32, mb2:mb2 + 2,
                                        half * 512:(half + 1) * 512],
                                start=True,
                                stop=True,
                                perf_mode=DR,
                                tile_position=(sub * 32, 0),
                            )
                        if EXP_PATTERN[kc] == "S":
                            nc.scalar.activation(
                                out=pr8[:, kc, :], in_=st[:, :],
                                func=AF.Exp, scale=0.125,
                            )
                        else:
                            nc.vector.tensor_scalar(
                                pr8u[:, kc, :], st[:, :],
                                float(SCHRAU_A), float(SCHRAU_B),
                                ALU.mult, ALU.add,
                            )
                    for half in range(2):
                        nc.tensor.matmul(
                            cx[:, half * 512:(half + 1) * 512],
                            lhsT=vaug[:, 2 * kp:2 * kp + 2, h, 0:65],
                            rhs=pr8[:, 2 * kp:2 * kp + 2,
                                    half * 512:(half + 1) * 512],
                            start=(kp == 0),
                            stop=(kp == 7),
                            perf_mode=DR,
                        )
                rec = work.tile([1, 1024], F32, tag="rec")
                nc.vector.reciprocal(rec[:, :], cx[64:65, :])
                recb = work.tile([64, 1024], F32, tag="recb")
                nc.gpsimd.partition_broadcast(recb[:, :], rec[:, :])
                nc.vector.tensor_tensor(
                    out=ctxT8[(h % 2) * 64:(h % 2) * 64 + 64, chn, (h % 4) // 2, :],
                    in0=cx[0:64, :],
                    in1=recb[:, :],
                    op=ALU.mult,
                )

            # ---------------- out proj + residual + LN ----------------
            for t8 in range(8):
                ps = ps_st.tile([P, 1024], F32, tag="st")
                for a in range(2):
                    nc.tensor.matmul(
                        ps[:, 0:512],
                        lhsT=ctxT8[:, a, :, t8 * P:(t8 + 1) * P],
                        rhs=wo_sb[:, a, :, :],
                        start=(a == 0),
                        stop=False,
                        perf_mode=DR,
                        skip_group_check=True,
                    )
                nc.tensor.matmul(
                    ps[:, 0:512],
                    lhsT=id_sb[:, :],
                    rhs=xres_sb[:, t8, :],
                    start=False,
                    stop=True,
                    skip_group_check=True,
                )
                stt6 = work.tile([P, 6], F32, tag="bn")
                nc.vector.bn_stats(out=stt6[:, :], in_=ps[:, 0:512])
                nc.vector.bn_aggr(out=mv_all[:, t8, :], in_=stt6[:, :])
                nc.scalar.activation(
                    out=y_sb[:, t8, :], in_=ps[:, 0:512], func=AF.Copy
                )
                nc.scalar.activation(
                    out=lnu[:, t8:t8 + 1], in_=mv_all[:, t8, 1:2],
                    func=AF.Ln, bias=eps_sb[:, :],
                )
                nc.scalar.activation(
                    out=rstd_all[:, t8:t8 + 1], in_=lnu[:, t8:t8 + 1],
                    func=AF.Exp, scale=-0.5,
                )
                eng = nc.vector  # TensorScalarPtr is not a legal Pool opcode on HW
                tmp = work.tile([P, D], F32, tag="lntmp")
                eng.scalar_tensor_tensor(
                    out=tmp[:, :],
                    in0=y_sb[:, t8, :],
                    scalar=mv_all[:, t8, 0:1],
                    in1=gm_sb[:, :],
                    op0=ALU.subtract,
                    op1=ALU.mult,
                )
                fin = work.tile([P, D], F32, tag="lnfin")
                eng.scalar_tensor_tensor(
                    out=fin[:, :],
                    in0=tmp[:, :],
                    scalar=rstd_all[:, t8:t8 + 1],
                    in1=bt_sb[:, :],
                    op0=ALU.mult,
                    op1=ALU.add,
                )
                nc.sync.dma_start(out=outd[t8 * P:(t8 + 1) * P, :], in_=fin[:, :])

    nc.compile()
    return nc


def _prep_shared(Wq, Wk, Wv, Wo, bq, bk, bv, bo, gamma, beta):
    """Host-side shared (per-weights) prep: permuted fp8 weights + biases."""
    e4 = ml_dtypes.float8_e4m3

    # Q/K output-row permutation: PE column j' = mb*128+po holds orig row
    # h*64+d with h = (mb//2)*4 + po//32, d = (mb%2)*32 + po%32.
    mbv, pov = np.meshgrid(np.arange(4), np.arange(128), indexing="ij")
    hh = (mbv // 2) * 4 + pov // 32
    dd = (mbv % 2) * 32 + pov % 32
    perm = (hh * 64 + dd).reshape(-1)

    def wsplit(w):  # [dout, din] -> [p, a, k, dout] fp8
        return np.ascontiguousarray(
            w.T.reshape(2, 2, 128, 512).transpose(2, 0, 1, 3)
        ).astype(e4)

    wq8 = wsplit(Wq[perm])
    wk8 = wsplit(Wk[perm])
    wv8 = wsplit(Wv)

    # out-proj: ctxT row (p, a, k) holds dv = (a*4 + k*2 + p//64)*64 + p%64
    pv, av, kv = np.meshgrid(
        np.arange(128), np.arange(2), np.arange(2), indexing="ij"
    )
    dvmap = (av * 4 + kv * 2 + pv // 64) * 64 + pv % 64
    wo8 = np.ascontiguousarray(Wo.T[dvmap].transpose(0, 1, 2, 3)).astype(e4)

    bq_p = np.ascontiguousarray(bq[perm].reshape(4, 128).T, dtype=np.float32)
    bk_p = np.ascontiguousarray(bk[perm].reshape(4, 128).T, dtype=np.float32)
    bo_eff = (bo + Wo @ bv).astype(np.float32)
    gm_n = np.ascontiguousarray(
        np.broadcast_to(gamma[None, :], (P, D)), dtype=np.float32)
    bt_n = np.ascontiguousarray(
        np.broadcast_to(beta[None, :], (P, D)), dtype=np.float32)
    id128 = np.eye(P).astype(ml_dtypes.bfloat16)
    return wq8, wk8, wv8, wo8, bq_p, bk_p, bo_eff, gm_n, bt_n, id128


def prepare_in_maps(x, Wq, bq, Wk, bk, Wv, bv, Wo, bo, gamma, beta):
    e4 = ml_dtypes.float8_e4m3
    x = np.asarray(x, np.float32)
    args = [np.asarray(v, np.float32)
            for v in (Wq, Wk, Wv, Wo, bq, bk, bv, bo, gamma, beta)]
    (wq8, wk8, wv8, wo8, bq_p, bk_p, bo_eff, gm_n, bt_n, id128) = _prep_shared(
        args[0], args[1], args[2], args[3], args[4], args[5], args[6],
        args[7], args[8], args[9])

    in_maps = []
    for c in range(NCORES):
        b = c // 2
        par = c % 2
        xt = x[b].T  # [D, S]
        xt8 = np.ascontiguousarray(
            xt.reshape(2, 2, 128, S).transpose(2, 0, 1, 3)).astype(e4)
        xtq8 = np.ascontiguousarray(
            xt[:, par * TQ:(par + 1) * TQ]
            .reshape(2, 2, 128, TQ).transpose(2, 0, 1, 3)).astype(e4)
        xres = (x[b, par * TQ:(par + 1) * TQ] + bo_eff[None, :]).astype(
            np.float32)
        xresb = np.ascontiguousarray(
            xres.reshape(8, 128, D).transpose(1, 0, 2)).astype(
            ml_dtypes.bfloat16)
        in_maps.append({
            "xt8": xt8, "xtq8": xtq8,
            "wq8": wq8, "wk8": wk8, "wv8": wv8, "wo8": wo8,
            "bqp": bq_p, "bkp": bk_p,
            "xres": xresb, "id128": id128,
            "gamma": gm_n, "beta": bt_n,
        })
    return in_maps


def _get_nc():
    global _NC_CACHE
    if _NC_CACHE is None:
        _NC_CACHE = _build()
    return _NC_CACHE


def kernel(x, Wq, bq, Wk, bk, Wv, bv, Wo, bo, gamma, beta):
    global LAST_RESULTS
    in_maps = prepare_in_maps(x, Wq, bq, Wk, bk, Wv, bv, Wo, bo, gamma, beta)
    nc = _get_nc()
    res = run_bass_kernel_spmd(
        nc, in_maps, core_ids=list(range(NCORES)), trace=TRACE)
    LAST_RESULTS = res

    outf = np.empty((B, S, D), np.float32)
    for c in range(NCORES):
        b = c // 2
        par = c % 2
        outf[b, par * TQ:(par + 1) * TQ, :] = res.results[c]["out"]
    return outf
